# revision 71
# baseline (speedup 1.0000x reference)
"""GQA kernel for Trainium2, 8 NeuronCores.

Problem: x[1,4096,2048], H=16 heads, G=4 kv-groups, D=128, causal mask,
RoPE on q/k, out-proj. Sharding: 2 heads per core (core c -> heads 2c,2c+1,
kv-group c//2). All tensors fed pre-transposed so matmul contractions land
on SBUF partitions. fp16 on-chip (PSUM accumulation fp32); exp is computed
as exp(s/sqrt(D) - 6*ln2) so weights fit fp16; the 2^-6 scale cancels in
the softmax normalization. (fp8 was evaluated offline: any e4m3 injection
point costs 2-4e-2 on the max-metric — over budget — and QK^T can't use
DoubleRow anyway since its contraction is only 128.)

Scheduling principles (derived from perfetto/ntff traces):
  - The PE clock is power-capped at 13/16 and drops to 4/8 after any
    >3.4us idle window, so every stall costs double: keep the in-order PE
    queue dense and never emit a PE instruction whose wait may be
    unsatisfied (waits are additionally hoisted to same-queue NOPs by
    _split_sync_waits).
  - The two HWDGE rings (sync, scalar) execute transfers serially and a
    gated DMA head-blocks the ring AND its engine: the xt stream owns the
    sync ring during projections, the exps own the scalar engine during
    attention, so collective-gated loads go on whichever ring is idle and
    only at points where their wait is provably satisfied (collectives
    measure ~25-40us trigger-to-done, serialized on the CC cores).
  - Collectives are merged: K/V pair-exchange as 4 pair AllGathers during
    projections (proj order [6,7,0,1,2,3,4,5] so the first pair lands
    mid-phase); ctx as 4 grouped AllGathers over attention-order pairs
    [[3,2],[6,1],[7,4],[5,0]].
  attn:  per-head causal attention in scoresT [k,q] orientation, order
         [3,2,6,1,7,4,5,0]; score matmuls run one group AHEAD of the exp;
         softmax denominator accumulated on DVE (GpSimd tensor ops are 3x
         slower), reduced by a ones-matmul, inverted by DVE reciprocal
         straight out of PSUM, partition-broadcast via SWDGE; the
         normalize chain lags one head and fires at g2/g4 so its serial
         DVE chain never blocks the PE queue.
  outp:  column-parallel out-proj, all 8 chunks as one dense PE block at
         the tail: (3,2,6,1) have satisfied collectives and cover the
         (7,4)/(5,0) exchanges that trigger at the schedule end.

PSUM is exactly 8 banks: ps(4) shared by proj/attn score tiles, pc(2) by
kv/ctx accumulators, scr(2) by norm reduce, out-proj accumulators and
V-transpose targets.

Output per core: outT_c = out.T[c*256:(c+1)*256, :]; host concatenates and
transposes back.
"""

import sys

for _p in ("/opt/trn_rl_repo",):
    if _p not in sys.path:
        sys.path.append(_p)

from contextlib import ExitStack

import numpy as np

import concourse.bass as bass
import concourse.tile as tile
from concourse import masks, mybir
from concourse.bass_utils import run_bass_kernel_spmd

F32 = mybir.dt.float32
F32R = mybir.dt.float32r
F16 = mybir.dt.float16
S = 4096
MAX_WAITS = 1  # walrus CoreV3 rejects instructions with more sync waits


def _split_sync_waits(nc, keep=0, per_nop=MAX_WAITS):
    """Hoist sem waits onto NOPs inserted before the instruction on the same
    engine queue (queue order makes this equivalent). keep=0 leaves the real
    instruction wait-free: a wait on a matmul breaks the PE's back-to-back
    pipelining (~145ns/matmul) even when already satisfied, while a satisfied
    wait on a NOP retires in ~20ns."""
    from concourse import mybir as mb
    n = 0
    for bassbb in nc.bb_map.values():
        bb = bassbb.bb
        insts = list(bb.instructions)
        out = []
        changed = False
        for ins in insts:
            si = ins.sync_info
            if si is not None and si.on_wait and len(si.on_wait) > keep:
                waits = list(si.on_wait)
                cut = len(waits) - keep
                head, rest = waits[:cut], waits[cut:]
                while head:
                    chunk, head = head[:per_nop], head[per_nop:]
                    n += 1
                    nop = mb.InstNoOp(
                        name=f"I-ws{n}",
                        engine=ins.engine,
                        ins=[],
                        outs=[],
                        sync_info=mb.SyncInfo(on_wait=chunk, on_update=[]),
                    )
                    nc.register_instruction(nop)
                    out.append(nop)
                ins.sync_info = mb.SyncInfo(
                    on_wait=rest, on_update=list(si.on_update or []))
                changed = True
            out.append(ins)
        if changed:
            try:
                bb.instructions[:] = out
            except TypeError:
                bb.set_instructions(out)
    return n


DIN = 2048
D = 128
HPC = 2          # heads per core
NCORES = 8
QC = 512         # q-chunk (free dim per matmul)
NQ = S // QC     # 8 q-chunks
KT = 128         # k tile (partition dim)
NKIN = DIN // 128  # 16 contraction tiles for projections
INV_SQRT_D = 1.0 / np.sqrt(D)
EXP_BIAS = float(-6.0 * np.log(2.0))  # 2^-6 scale on exp; cancels in softmax


def build_nc():
    nc = bass.Bass(num_devices=NCORES)

    xT = nc.dram_tensor("xT", [DIN, S], F16, kind="ExternalInput")
    wqT = nc.dram_tensor("wqT", [DIN, HPC * D], F16, kind="ExternalInput")
    # K weights on even cores, V weights on odd cores (pair-exchanged)
    wkvT = nc.dram_tensor("wkvT", [DIN, D], F16, kind="ExternalInput")
    cosT = nc.dram_tensor("cosT", [D, S], F16, kind="ExternalInput")
    sinT = nc.dram_tensor("sinT", [D, S], F16, kind="ExternalInput")
    woT = nc.dram_tensor("woT", [DIN, HPC * D], F16, kind="ExternalInput")
    outT = nc.dram_tensor("outT", [HPC * D, S], F32, kind="ExternalOutput")

    with ExitStack() as ctx:
        tc = ctx.enter_context(tile.TileContext(nc))

        res = ctx.enter_context(tc.tile_pool(name="res", bufs=1))
        dram = ctx.enter_context(tc.tile_pool(name="dram", bufs=1, space="DRAM"))

        # collective bounce tiles (dependency-tracked DRAM tiles; outputs in
        # the Shared segment so the runtime takes the direct HBM-HBM path).
        # ctx AllGathers are grouped: collective latency is ~20us regardless
        # of size, so pairs amortize it; the schedule ends on two singleton
        # groups so the tail only ever waits on one small exchange.
        GRPS = [[3, 2], [6, 1], [7, 4], [5, 0]]
        GMAP = {qc: (g, s) for g, grp in enumerate(GRPS)
                for s, qc in enumerate(grp)}
        ctx_loc = [dram.tile([HPC * D, len(grp) * QC], F16, tag=f"cl{g}",
                             name=f"cl{g}")
                   for g, grp in enumerate(GRPS)]
        ctx_ful = [dram.tile([NCORES * HPC * D, len(grp) * QC], F16,
                             tag=f"cf{g}", name=f"cf{g}", addr_space="Shared")
                   for g, grp in enumerate(GRPS)]
        # K/V pair-exchange bounce tiles (core pair shares one kv-group;
        # even core projects K, odd core projects V, AllGather over pairs;
        # two q-chunks merged per collective to amortize rendezvous latency)
        kv_loc = [dram.tile([D, 2 * QC], F16, tag=f"kl{p}", name=f"kl{p}")
                  for p in range(NQ // 2)]
        kv_pair = [dram.tile([2 * D, 2 * QC], F16, tag=f"kp{p}", name=f"kp{p}")
                   for p in range(NQ // 2)]

        # resident SBUF tensors
        qt = res.tile([128, HPC, S], F16, tag="qt")          # QT per head
        kt = res.tile([128, S], F16, tag="kt")               # KT (shared group)
        vt = res.tile([128, S // 128, D], F16, tag="vt")     # V as s-tiles
        # W_q / W_kv split into three tiles: tile-granular dependency
        # tracking would otherwise make the first projection matmul wait
        # for ALL pieces of a single wq tile (measured 12us startup stall).
        # The first piece is 2 contraction tiles so the ki=0 matmul gates
        # on only ~320KB of cold-HBM traffic.
        wq_a = res.tile([128, 2, HPC * D], F16, tag="wqa")
        wq_b = res.tile([128, 6, HPC * D], F16, tag="wqb")
        wq_c = res.tile([128, 8, HPC * D], F16, tag="wqc")
        wkv_a = res.tile([128, 2, D], F16, tag="wkva")
        wkv_b = res.tile([128, 6, D], F16, tag="wkvb")
        wkv_c = res.tile([128, 8, D], F16, tag="wkvc")
        wo_sb = res.tile([128, NKIN, HPC * D], F16, tag="wo")

        def wq_t(ki):
            if ki < 2:
                return wq_a[:, ki, :]
            return wq_b[:, ki - 2, :] if ki < 8 else wq_c[:, ki - 8, :]

        def wkv_t(ki):
            if ki < 2:
                return wkv_a[:, ki, :]
            return wkv_b[:, ki - 2, :] if ki < 8 else wkv_c[:, ki - 8, :]
        cos_sb = res.tile([128, S], F16, tag="cos")
        sin_sb = res.tile([128, S], F16, tag="sin")
        ones_k = res.tile([128, 1], F16, tag="ones_k")       # lhsT for col sums
        ebias = res.tile([128, 1], F32, tag="ebias")         # exp bias 2^-6
        ones_r = res.tile([1, 128], F16, tag="ones_r")       # lhsT for bcast
        ident = res.tile([128, 128], F16, tag="ident")       # PE transpose id

        nc.vector.memset(ones_k, 1.0)
        nc.vector.memset(ebias, EXP_BIAS)
        nc.vector.memset(ones_r, 1.0)
        masks.make_identity(nc, ident)

        # weight loads: first pieces only, so chunk-0 projection matmuls start
        # ~3us in instead of waiting ~17us for 5 serialized full-tensor DMAs.
        # Remaining pieces are emitted inside the chunk-0 loop (after its xt
        # batches), cos/sin/wo go on the scalar HWDGE ring in parallel.
        wqT_r = wqT.rearrange("(t p) m -> p t m", p=128)
        wkvT_r = wkvT.rearrange("(t p) m -> p t m", p=128)
        nc.sync.dma_start(out=wq_a, in_=wqT_r[:, 0:2, :])
        nc.sync.dma_start(out=wkv_a, in_=wkvT_r[:, 0:2, :])
        # scalar-ring loads ordered by first use (wq_b at ki=2, wq_c at
        # ki=8, cos/sin at the first rope); wo is deferred into the proj
        # loop so the startup HBM crunch doesn't delay the sync ring
        nc.scalar.dma_start(out=wq_b, in_=wqT_r[:, 2:8, :])
        nc.scalar.dma_start(out=wkv_b, in_=wkvT_r[:, 2:8, :])
        nc.scalar.dma_start(out=wq_c, in_=wqT_r[:, 8:16, :])
        nc.scalar.dma_start(out=wkv_c, in_=wkvT_r[:, 8:16, :])
        nc.scalar.dma_start(out=cos_sb, in_=cosT[:, :])
        nc.scalar.dma_start(out=sin_sb, in_=sinT[:, :])

        # SBUF pools
        xpool = ctx.enter_context(tc.tile_pool(name="xpool", bufs=2))
        rpool = ctx.enter_context(tc.tile_pool(name="rope", bufs=3))
        kvpool = ctx.enter_context(tc.tile_pool(name="kvp", bufs=2))
        wpool = ctx.enter_context(tc.tile_pool(name="wpool", bufs=4))
        apool = ctx.enter_context(tc.tile_pool(name="acc", bufs=2))
        npool = ctx.enter_context(tc.tile_pool(name="norm", bufs=2))
        copool = ctx.enter_context(tc.tile_pool(name="cout", bufs=2))
        cpool = ctx.enter_context(tc.tile_pool(name="cpool", bufs=2))
        opool = ctx.enter_context(tc.tile_pool(name="opool", bufs=2))
        # PSUM: exactly 8 banks. "ps" serves proj q-accumulators AND attn
        # score tiles; "pc" serves proj kv-accumulator AND attn ctx
        # accumulator; "scr" serves softmax aux/broadcast, out-proj
        # accumulators and the V-transpose target.
        ps_pool = ctx.enter_context(tc.tile_pool(name="ps", bufs=2, space="PSUM"))
        pc_pool = ctx.enter_context(tc.tile_pool(name="pc", bufs=2, space="PSUM"))
        scr_pool = ctx.enter_context(tc.tile_pool(name="scr", bufs=2, space="PSUM"))

        def rope(dst, src, cos_c, sin_c):
            # dst = src*cos + rot(src)*sin, rotate-half along partitions
            rot = rpool.tile([128, QC], F16, tag="rot", name="rot")
            nc.vector.tensor_scalar_mul(rot[0:64, :], src[64:128, :], -1.0)
            nc.vector.tensor_copy(rot[64:128, :], src[0:64, :])
            nc.vector.tensor_mul(dst, src, cos_c)
            nc.vector.tensor_mul(rot, rot, sin_c)
            nc.vector.tensor_add(dst, dst, rot)

        def ingest_kv(qc, eng=None):
            # read back the pair-gathered K/V chunk: rows 0:128 = KT
            # (even core's projection), rows 128:256 = VT (odd core's).
            # Ring choice: during projections the scalar ring is idle and
            # the sync ring carries the xt stream; during attention the
            # scalar ring carries the exps (a DMA issue occupies the engine
            # for the whole transfer) and the sync ring is nearly free.
            eng = eng or nc.sync
            q0 = qc * QC
            p, half = qc // 2, (qc % 2) * QC
            kb = kvpool.tile([128, QC], F16, tag="kb", name="kb")
            vb = kvpool.tile([128, QC], F16, tag="vb", name="vb")
            eng.dma_start(out=kb, in_=kv_pair[p][0:D, half:half + QC])
            eng.dma_start(out=vb, in_=kv_pair[p][D:2 * D, half:half + QC])
            rope(kt[:, q0:q0 + QC], kb,
                 cos_sb[:, q0:q0 + QC], sin_sb[:, q0:q0 + QC])
            pv2 = scr_pool.tile([128, 4, D], F16, tag="scr", name="pv2")
            for si in range(4):
                nc.tensor.transpose(
                    pv2[:, si, :], vb[:, si * 128:(si + 1) * 128], ident)
            nc.vector.tensor_copy(vt[:, qc * 4:(qc + 1) * 4, :], pv2)

        # ---------------- projections + RoPE + K/V pair exchange -------------
        # Projection chunk order starts with the pair (6,7): its pair
        # collective then completes mid-projections instead of right at
        # attention start, where its ingest's PE transposes (gated on the
        # collective) head-blocked the in-order PE queue for ~8us.
        PORD = [6, 7, 0, 1, 2, 3, 4, 5]
        # proj position j -> [(ki, kv chunk)] ingest slots. Collective
        # trigger-to-done jitters 8-40us run to run, so every ingest trails
        # its pair collective by >=35us; chunks 0..5,7 land just before /
        # inside early attention (kv 7 isn't consumed until the 5th
        # attention chunk).
        ING = {7: [(4, 6)]}
        xT_r = xT.rearrange("(t p) m -> p t m", p=128)
        for j in range(NQ):
            qc = PORD[j]
            q0 = qc * QC
            pq = ps_pool.tile([128, 2, QC], F32, tag="ps", name="pq")
            pkv = pc_pool.tile([128, QC], F32, tag="pc", name="pkv")
            # two batched xt loads per chunk (8 contraction tiles each): one
            # DMA issue (~0.6us) instead of 8 on the in-order sync ring.
            # The first chunk's first batch is split so the ki=0 matmuls
            # start after 256KB instead of 1MB of cold-HBM streaming.
            xtb = [None] * NKIN
            for (lo, hi) in [(0, 1), (1, 8), (8, 16)]:
                xt_t = xpool.tile([128, hi - lo, QC], F16, tag=f"xt{lo}",
                                  name=f"xt{qc}p{lo}")
                nc.sync.dma_start(
                    out=xt_t, in_=xT_r[:, lo:hi, q0:q0 + QC])
                for ki in range(lo, hi):
                    xtb[ki] = (xt_t, ki - lo)
            if j == 2:
                nc.scalar.dma_start(
                    out=wo_sb, in_=woT.rearrange("(t p) m -> p t m", p=128))
            for ki in range(NKIN):
                xt_t, sub = xtb[ki]
                xt = xt_t[:, sub, :]
                st = ki == 0
                sp = ki == NKIN - 1
                for h in range(HPC):
                    nc.tensor.matmul(
                        pq[:, h, :], lhsT=wq_t(ki)[:, h * D:(h + 1) * D],
                        rhs=xt, start=st, stop=sp)
                nc.tensor.matmul(pkv, lhsT=wkv_t(ki), rhs=xt,
                                 start=st, stop=sp)
                for (slot, kv) in ING.get(j, []):
                    if ki == slot:
                        ingest_kv(kv, eng=nc.scalar)
            kvout = kvpool.tile([128, QC], F16, tag="kvout", name="kvout")
            nc.vector.tensor_copy(kvout, pkv)
            half = (qc % 2) * QC
            nc.sync.dma_start(out=kv_loc[qc // 2][:, half:half + QC],
                              in_=kvout)
            if qc % 2 == 1:
                nc.gpsimd.collective_compute(
                    "AllGather",
                    mybir.AluOpType.bypass,
                    replica_groups=[[2 * p, 2 * p + 1]
                                    for p in range(NCORES // 2)],
                    ins=[kv_loc[qc // 2].opt()],
                    outs=[kv_pair[qc // 2].opt()],
                )
            for h in range(HPC):
                rope(qt[:, h, q0:q0 + QC], pq[:, h, :],
                     cos_sb[:, q0:q0 + QC], sin_sb[:, q0:q0 + QC])

        # ---------------- attention + out-proj ----------------
        # Pending normalize work, lagged one head so its serial ACT chain
        # hides behind the next head's matmuls.
        pend_norm = [None, None]  # [denominator_fn, rest_fn]
        norm_state = {}

        def make_norm_a(qc, h, acc, acc2, pc):
            def fire():
                # cross-partition reduce of the exp sums on the PE, then
                # 1/denom on the DVE straight out of PSUM (no bounce copy).
                # The unnormalized ctx is copied out of PSUM here too: that
                # releases the pc bank ~2 groups earlier than the old
                # end-of-norm-chain release, whose lateness stalled the
                # next-next head's ctx accumulation 3-5us per head (and
                # each such stall risks a ~17us HAM half-clock window).
                aux = scr_pool.tile([128, QC], F32, tag="scr", name="aux")
                nc.tensor.matmul(aux[0:1, :], lhsT=ones_k, rhs=acc,
                                 start=True, stop=False)
                nc.tensor.matmul(aux[0:1, :], lhsT=ones_k, rhs=acc2,
                                 start=False, stop=True)
                rec32 = npool.tile([1, QC], F32, tag="rec32", name="rec32")
                nc.vector.reciprocal(rec32, aux[0:1, :])
                pcs = copool.tile([128, QC], F16, tag="pcs", name="pcs")
                nc.vector.tensor_copy(pcs, pc)
                norm_state[(qc, h)] = (rec32, pcs)
            return fire

        def make_norm_b(qc, h):
            def fire():
                rec32, pcs = norm_state.pop((qc, h))
                # broadcast 1/denom across partitions via a ones-matmul
                # (SWDGE partition-broadcast fails codegen on this stack)
                rec16 = npool.tile([1, QC], F16, tag="rec16", name="rec16")
                nc.vector.tensor_copy(rec16, rec32)
                pb = scr_pool.tile([128, QC], F32, tag="scr", name="pb")
                nc.tensor.matmul(pb, lhsT=ones_r, rhs=rec16,
                                 start=True, stop=True)
                bc = npool.tile([128, QC], F32, tag="bc", name="bc")
                nc.vector.tensor_copy(bc, pb)
                # final multiply on GpSimd (~idle during attention; it
                # cannot read PSUM, hence the SBUF ctx bounce above)
                cout = copool.tile([128, QC], F16, tag="cout", name="cout")
                nc.gpsimd.tensor_mul(cout, pcs, bc)
                g, side = GMAP[qc]
                nc.sync.dma_start(
                    out=ctx_loc[g][h * D:(h + 1) * D,
                                   side * QC:(side + 1) * QC], in_=cout)
                if h == HPC - 1 and qc == GRPS[g][-1]:
                    nc.gpsimd.collective_compute(
                        "AllGather",
                        mybir.AluOpType.bypass,
                        replica_groups=[list(range(NCORES))],
                        ins=[ctx_loc[g].opt()],
                        outs=[ctx_ful[g].opt()],
                    )
            return fire

        def attn_chunk(qc, extra=None):
            # extra: dict {g: callable} fired after group g's score matmuls
            extra = dict(extra or {})
            q0 = qc * QC
            nk = (qc + 1) * 4
            ng = nk // 2
            for h in range(HPC):
                pc = pc_pool.tile([128, QC], F32, tag="pc", name="pc")
                acc = apool.tile([128, QC], F16, tag="acc", name="acc")
                acc2 = apool.tile([128, QC], F16, tag="acc2", name="acc2")
                prev = None  # (wt, g) waiting for its pc matmuls

                def consume(wt, g):
                    # pc matmuls for group g (lagged one group behind the
                    # score matmuls so the PE never waits on exp). Diagonal
                    # tiles skip their fully-masked leading columns (zeros in
                    # wt); the ragged accumulation trips the sim's group
                    # check, which is safe to skip here.
                    for j in range(2):
                        ki = 2 * g + j
                        trim = (ki - 4 * qc) * KT if ki >= 4 * qc else 0
                        nc.tensor.matmul(pc[:, trim:], lhsT=vt[:, ki, :],
                                         rhs=wt[:, j, trim:],
                                         start=(ki == 0), stop=(ki == nk - 1),
                                         skip_group_check=True)

                for g in range(ng):
                    ps = ps_pool.tile([128, 2, QC], F32, tag="ps", name="ps")
                    wt = wpool.tile([128, 2, QC], F16, tag="wt", name="wt")
                    for j in range(2):
                        ki = 2 * g + j
                        k0 = ki * KT
                        # columns [0, trim) of a diagonal tile are fully
                        # masked: skip them (affine_select zero-fills them in
                        # wt, so the stale-PSUM exp there is never consumed)
                        trim = (ki - 4 * qc) * KT if ki >= 4 * qc else 0
                        nc.tensor.matmul(ps[:, j, trim:],
                                         lhsT=kt[:, k0:k0 + KT],
                                         rhs=qt[:, h, q0 + trim:q0 + QC],
                                         start=True, stop=True)
                    # prev head's normalize fires at g1/g3 (not g0/g1): the
                    # broadcast matmul sits in the in-order PE queue at its
                    # emission point, and firing too early makes the PE wait
                    # ~4us on the serial DVE reciprocal chain feeding it
                    if g == min(2, ng - 1) and pend_norm[0] is not None:
                        pend_norm[0]()  # prev head's denominator reduce
                        pend_norm[0] = None
                    if prev is not None:
                        consume(*prev)
                    if g == min(4, ng - 1) and pend_norm[1] is not None:
                        pend_norm[1]()  # rest of prev head's normalize
                        pend_norm[1] = None
                    if h == 0 and g in extra:
                        extra.pop(g)()  # deferred kv ingests
                    nc.scalar.activation(wt, ps,
                                         mybir.ActivationFunctionType.Exp,
                                         scale=INV_SQRT_D, bias=ebias[:, :])
                    for j in range(2):
                        k0 = (2 * g + j) * KT
                        if k0 + KT - 1 > q0:
                            # keep where (q0+col) - (k0+p) >= 0
                            nc.gpsimd.affine_select(
                                out=wt[:, j, :], in_=wt[:, j, :],
                                pattern=[[1, QC]],
                                compare_op=mybir.AluOpType.is_ge, fill=0.0,
                                base=q0 - k0, channel_multiplier=-1)
                    # softmax denominator partials, both on DVE: GpSimd
                    # tensor ops are 3x slower for the same [128,512] add
                    if g == 0:
                        nc.vector.tensor_copy(acc, wt[:, 0, :])
                        nc.vector.tensor_copy(acc2, wt[:, 1, :])
                    else:
                        nc.vector.tensor_add(acc, acc, wt[:, 0, :])
                        nc.vector.tensor_add(acc2, acc2, wt[:, 1, :])
                    prev = (wt, g)
                consume(*prev)
                for fn in pend_norm:
                    if fn is not None:
                        fn()
                pend_norm[0] = make_norm_a(qc, h, acc, acc2, pc)
                pend_norm[1] = make_norm_b(qc, h)

        def outp_load(qc):
            # gathered-ctx readback as 2 batched DMAs on the scalar ring
            # (gated on the group's AllGather; keeps the sync ring free)
            g, side = GMAP[qc]
            full_r = ctx_ful[g].rearrange("(t p) m -> p t m", p=128)
            cts = []
            for b in range(2):
                ct = cpool.tile([128, 8, QC], F16, tag=f"ctb{b}",
                                name=f"ct{qc}b{b}")
                nc.scalar.dma_start(
                    out=ct,
                    in_=full_r[:, b * 8:(b + 1) * 8,
                               side * QC:(side + 1) * QC])
                cts.append(ct)
            return cts

        def outp_compute(qc, cts):
            for m in range(HPC):
                po = scr_pool.tile([128, QC], F32, tag="scr", name="po")
                for ti in range(NKIN):
                    nc.tensor.matmul(
                        po, lhsT=wo_sb[:, ti, m * D:(m + 1) * D],
                        rhs=cts[ti // 8][:, ti % 8, :],
                        start=(ti == 0), stop=(ti == NKIN - 1))
                ot = opool.tile([128, QC], F32, tag="ot", name="ot")
                nc.vector.tensor_copy(ot, po)
                nc.sync.dma_start(
                    out=outT[m * 128:(m + 1) * 128, qc * QC:(qc + 1) * QC],
                    in_=ot)

        def outp_chunk(qc):
            outp_compute(qc, outp_load(qc))

        # Attention order: small/big interleave for norm-chain balance, big
        # chunks mid-schedule so their grouped ctx collectives hide under
        # later attention, tiny chunk 0 last so the final exchange is small.
        # kv ingests hook into early attention groups, each ~30us+ after
        # its pair collective fired (first-collective latency ~35us).
        order = [3, 2, 6, 1, 7, 4, 5, 0]
        ingest_kv(0)  # pair collective (0,1) completed mid-projections
        ingest_kv(1)
        hooks = {0: {1: lambda: ingest_kv(2), 3: lambda: ingest_kv(3),
                     5: lambda: ingest_kv(7)},
                 2: {1: lambda: ingest_kv(4), 3: lambda: ingest_kv(5)}}
        # outp(3)/outp(2) run in-loop at positions where their collective
        # is ~80us satisfied: halves the tail's ctx-readback HBM burst,
        # which was stretching the final (5,0) exchange to ~50us
        outp_at = {5: [3], 6: [2]}
        for i, qc in enumerate(order):
            attn_chunk(qc, extra=hooks.get(i))
            for oc in outp_at.get(i, []):
                outp_chunk(oc)
        for fn in pend_norm:
            if fn is not None:
                fn()
        pend_norm[0] = pend_norm[1] = None
        # tail: (6,1) have long-satisfied collectives and cover the (7,4)
        # and (5,0) exchanges that trigger near the schedule end
        tail_cts = {qc: outp_load(qc) for qc in (6, 1)}
        for qc in (6, 1):
            outp_compute(qc, tail_cts[qc])
        for qc in (7, 4, 5, 0):
            outp_compute(qc, outp_load(qc))

    _split_sync_waits(nc)
    return nc



_NC_CACHE = None


def _get_nc():
    global _NC_CACHE
    if _NC_CACHE is None:
        _NC_CACHE = build_nc()
    return _NC_CACHE


def _make_in_maps(x, cos, sin, Wq, Wk, Wv, Wo):
    f16 = np.float16
    xT = np.ascontiguousarray(x.reshape(S, DIN).T.astype(f16))
    cosT = np.ascontiguousarray(cos.T.astype(f16))
    sinT = np.ascontiguousarray(sin.T.astype(f16))
    in_maps = []
    for c in range(NCORES):
        g = c // 2
        wkv = Wk if c % 2 == 0 else Wv
        in_maps.append({
            "xT": xT,
            "wqT": np.ascontiguousarray(Wq[c * 256:(c + 1) * 256, :].T.astype(f16)),
            "wkvT": np.ascontiguousarray(
                wkv[g * 128:(g + 1) * 128, :].T.astype(f16)),
            "cosT": cosT,
            "sinT": sinT,
            "woT": np.ascontiguousarray(Wo[c * 256:(c + 1) * 256, :].T.astype(f16)),
        })
    return in_maps


def run(x, cos, sin, Wq, Wk, Wv, Wo, trace=False):
    nc = _get_nc()
    in_maps = _make_in_maps(x, cos, sin, Wq, Wk, Wv, Wo)
    res = run_bass_kernel_spmd(nc, in_maps, list(range(NCORES)), trace=trace)
    outT = np.concatenate([res.results[c]["outT"] for c in range(NCORES)], axis=0)
    out = np.ascontiguousarray(outT.T).reshape(1, S, DIN).astype(np.float32)
    return out, res


def kernel(x, mask, cos, sin, Wq, Wk, Wv, Wo):
    out, _ = run(np.asarray(x, dtype=np.float32), np.asarray(cos, np.float32),
                 np.asarray(sin, np.float32), np.asarray(Wq, np.float32),
                 np.asarray(Wk, np.float32), np.asarray(Wv, np.float32),
                 np.asarray(Wo, np.float32))
    return out

